# revision 3
# baseline (speedup 1.0000x reference)
"""Multi-head causal attention (B=4, T=2048, D=1024, H=16, d_k=64) on 8 trn2 cores.

Sharding: 8 cores = 4 batches x 2 head-groups (8 heads each). Per core:
Q^T/K^T projections in [c, t] layout, V in natural [s, c] layout with an
appended ones column. Scores are computed transposed (scores^T[s, t],
d_k=64 contraction split across the two PE row-halves via tile_position)
and exp'd on ACT; the causal boundary triangle is zeroed by a DVE
multiply with a constant 0/1 tile.

attn@V uses the exp'd scores tile E[s, t-128] as the PE *stationary*
operand and V-hat = [V | ones] as the 65-row *moving* operand, producing
context^T [t, 65] per head at full 128-partition output width. The
baseline layout (V stationary, E moving) only filled 65 of 128 output
partitions, so each moving row did half work; this halves attn@V PE
occupancy (HW A/B: 0.90x whole-kernel). The ones column lands the
softmax denominator Z in psum column 64, replacing the baseline's
Z spread/reciprocal/DRAM-bounce-broadcast chain with one [128, 2] DVE
reciprocal plus a fused psum*(1/Z)->bf16 multiply per (j, t-tile).

PSUM discipline: start=True clears the has_written bits of the WHOLE
bank, so each per-j context bank ([128,512] fp32, regions at
(tt%2)*130 + h*65) forms a single accumulation group: only the
chronologically first matmul into the bank carries start, only the last
carries stop. Banks: pp 2 + pS (scores/outproj) 4 + pC 2 = 8.

The c-hat [t, (j,h,c)] tiles are transposed back to [c, t] for the
out-projection by the XBAR transpose DMA (16x128 hw tiles, out
partition = c' % 128, middle dim = c' // 128 - verified on HW) on the
ACT HWDGE queue, which this variant leaves otherwise idle.

Measured (interleaved A/B slope, contended phase): 267.7-274.7 us/iter
vs baseline 295.8-298.6 us/iter (ratio 0.90-0.92 across 6 rounds).
Rel err 5.42e-3 (gate 2e-2), identical to baseline.
"""

import sys

if "/opt/trn_rl_repo" not in sys.path:
    sys.path.insert(0, "/opt/trn_rl_repo")

from contextlib import ExitStack

import ml_dtypes
import numpy as np

import concourse.bass as bass
import concourse.bacc as bacc
import concourse.mybir as mybir
import concourse.tile as tile
from concourse.bass_utils import run_bass_kernel_spmd

D = 1024  # model dim
C = 512   # per-core projection cols (8 heads x 64)
NJ = 4    # head-pair chunks of 128 channels
NKC = 8   # contraction chunks of 128 over D
DT = mybir.dt.float32
FR = mybir.dt.float32r
BF = mybir.dt.bfloat16
EXP = mybir.ActivationFunctionType.Exp


def build_nc(T=2048, loop_reps=1, ablate=(), flush_depth=3, ep_bufs=10,
             xp_bufs=5, op_bufs=4, qtp_bufs=2, cxp_bufs=2):
    NR = T // 512
    ablate = set(ablate)

    nc = bacc.Bacc("TRN2", target_bir_lowering=False, debug=False)
    xr_d = nc.dram_tensor("xr", [128, NR, NKC, 512], BF, kind="ExternalInput").ap()
    wq_d = nc.dram_tensor("wq", [128, NKC, C], BF, kind="ExternalInput").ap()
    wk_d = nc.dram_tensor("wk", [128, NKC, C], BF, kind="ExternalInput").ap()
    wv_d = nc.dram_tensor("wv", [128, NKC, C], BF, kind="ExternalInput").ap()
    wo_d = nc.dram_tensor("wo", [128, NJ, D], BF, kind="ExternalInput").ap()
    out_d = nc.dram_tensor("out", [T, D], BF, kind="ExternalOutput").ap()

    with tile.TileContext(nc) as tc, ExitStack() as ctx:
        main = ctx.enter_context(tc.tile_pool(name="main", bufs=1))

        wq_s = main.tile([128, NKC, C], BF, tag="wq")
        wk_s = main.tile([128, NKC, C], BF, tag="wk")
        wv_s = main.tile([128, NKC, C], BF, tag="wv")
        wo_s = main.tile([128, NJ, D], BF, tag="wo")
        for hf in range(2):
            nc.sync.dma_start(wq_s[:, hf * 4:(hf + 1) * 4, :],
                              wq_d[:, hf * 4:(hf + 1) * 4, :])
        for hf in range(2):
            nc.sync.dma_start(wk_s[:, hf * 4:(hf + 1) * 4, :],
                              wk_d[:, hf * 4:(hf + 1) * 4, :])
        for hf in range(2):
            nc.sync.dma_start(wv_s[:, hf * 4:(hf + 1) * 4, :],
                              wv_d[:, hf * 4:(hf + 1) * 4, :])
        nc.sync.dma_start(wo_s[:], wo_d[:])

        # constant causal-boundary mask: tri[p, h, f] = 1 if f >= p else 0
        tri = main.tile([128, 2, 128], BF, tag="tri")
        nc.vector.memset(tri[:], 1.0)
        nc.gpsimd.affine_select(
            out=tri[:], in_=tri[:], compare_op=mybir.AluOpType.is_ge,
            fill=0.0, base=0, pattern=[[0, 2], [1, 128]], channel_multiplier=-1)

        if loop_reps > 1:
            ctx.enter_context(tc.For_i(
                0, loop_reps, 1, staggered_reset=True,
                hint_engines=(mybir.EngineType.PE, mybir.EngineType.Activation,
                              mybir.EngineType.DVE, mybir.EngineType.Pool,
                              mybir.EngineType.SP)))
        qtp = ctx.enter_context(tc.tile_pool(name="qtp", bufs=qtp_bufs))
        cxp = ctx.enter_context(tc.tile_pool(name="cxp", bufs=cxp_bufs))
        chp = ctx.enter_context(tc.tile_pool(name="chp", bufs=8))
        xp = ctx.enter_context(tc.tile_pool(name="xp", bufs=xp_bufs))
        ep = ctx.enter_context(tc.tile_pool(name="ep", bufs=ep_bufs))
        zp = ctx.enter_context(tc.tile_pool(name="zp", bufs=8))
        op = ctx.enter_context(tc.tile_pool(name="op", bufs=op_bufs))
        pp = ctx.enter_context(tc.tile_pool(name="pp", bufs=2, space="PSUM"))
        pS = ctx.enter_context(tc.tile_pool(name="pS", bufs=2, space="PSUM"))
        pC = ctx.enter_context(tc.tile_pool(name="pC", bufs=2, space="PSUM"))

        xhs = {}

        def load_x(r):
            halves = []
            for hf in range(2):
                xh = xp.tile([128, 4, 512], BF, tag="xt", name=f"xh{r}_{hf}")
                nc.sync.dma_start(xh[:], xr_d[:, r, hf * 4:(hf + 1) * 4, :])
                halves.append(xh)
            xhs[r] = halves

        load_x(0)

        kts = []   # per-r K^T tiles [128, NJ, 512]
        vts = []   # per-r V tiles [128, 4, 8, 65] (s-tiles 4r..4r+3)
        qts = {}
        cxs = {}

        def proj_chunks(r):
            if r not in xhs:
                load_x(r)
            halves = xhs[r]

            def xchunk(kc):
                return halves[kc // 4][:, kc % 4, :]

            qt = qtp.tile([128, NJ, 512], BF, tag="qt", name=f"qt{r}")
            kt = main.tile([128, NJ, 512], BF, tag=f"kt{r}", name=f"kt{r}")
            vt = main.tile([128, 4, 8, 65], BF, tag=f"vt{r}", name=f"vt{r}")
            qts[r] = qt
            kts.append(kt)
            vts.append(vt)
            nc.vector.memset(vt[:, :, :, 64:65], 1.0)
            groups = []

            def qk_group(w_s, dst, j, tag2):
                def emit():
                    ps = pp.tile([128, 512], DT, tag="pp",
                                 name=f"psqk{r}_{j}_{tag2}")
                    if "projmm" not in ablate:
                        for kc in range(NKC):
                            nc.tensor.matmul(
                                ps[:], w_s[:, kc, j * 128:(j + 1) * 128],
                                xchunk(kc),
                                start=(kc == 0), stop=(kc == NKC - 1))
                    else:
                        nc.tensor.matmul(
                            ps[:, 0:8], w_s[:, 0, j * 128:(j + 1) * 128],
                            xchunk(0)[:, 0:8], start=True, stop=True)
                    with nc.allow_low_precision(reason="bf16 store"):
                        nc.vector.tensor_copy(dst[:, j, :], ps[:])
                return emit

            def v_group(al):
                def emit():
                    ps = pp.tile([128, 512], DT, tag="pp", name=f"psv{r}_{al}")
                    if "projmm" not in ablate:
                        for kc in range(NKC):
                            nc.tensor.matmul(
                                ps[:], xchunk(kc)[:, al * 128:(al + 1) * 128],
                                wv_s[:, kc, :],
                                start=(kc == 0), stop=(kc == NKC - 1))
                    else:
                        nc.tensor.matmul(ps[:, 0:8],
                                         xchunk(0)[:, al * 128:(al + 1) * 128],
                                         wv_s[:, 0, 0:8], start=True, stop=True)
                    with nc.allow_low_precision(reason="bf16 store"):
                        nc.vector.tensor_copy(
                            vt[:, al, :, 0:64],
                            ps[:].rearrange("p (h e) -> p h e", h=8))
                return emit

            for w_s, dst, tag2 in ((wq_s, qt, 0), (wk_s, kt, 1)):
                for j in range(NJ):
                    groups.append(qk_group(w_s, dst, j, tag2))
            for al in range(4):
                groups.append(v_group(al))
            return groups

        def proj(r):
            for g in proj_chunks(r):
                g()

        def attn(b, filler=None):
            na = 4 * b + 4
            qt = qts[b]
            # c-hat per t-tile of this 512-range: [t 128, (j, h, c) 512] bf16
            chs = [chp.tile([128, 512], BF, tag="ch", name=f"ch{b}_{tt}")
                   for tt in range(4)]
            cx = cxp.tile([128, NJ, 512], BF, tag="cx", name=f"cx{b}")
            cxs[b] = cx
            for j in range(NJ):
                if j > 0 and filler is not None:
                    filler(j)
                # per-j context^T psum: two full-bank [128, 512] tiles
                # (tt 0-1 and tt 2-3), regions at (tt%2)*130 + h*65.
                # ONE accumulation group per bank: start=True clears the
                # whole bank's has_written bits, so only the chronologically
                # first matmul into each bank may carry start, and only the
                # last carries stop.
                psc = [pC.tile([128, 512], DT, tag="psc",
                               name=f"psc{b}_{j}_{g}") for g in range(2)]
                pend = []

                def flush(n, pend=pend, psc=psc, j=j, na=na):
                    while len(pend) > n:
                        a0, do0, et0 = pend.pop(0)
                        et0_v = et0[:].rearrange("p (h f) -> p h f", h=2)
                        tt_lo = max(0, a0 - 4 * b)
                        for tt in range(tt_lo, 4):
                            g = tt // 2
                            col = (tt % 2) * 130
                            first = (a0 == 0 and tt == 2 * g)
                            last = (a0 == 4 * b + tt and tt == 2 * g + 1)
                            for h0 in range(2):
                                if "attnv" not in ablate:
                                    nc.tensor.matmul(
                                        psc[g][:, col + h0 * 65:
                                               col + (h0 + 1) * 65],
                                        et0_v[:, h0, tt * 128:(tt + 1) * 128],
                                        vts[a0 // 4][:, a0 % 4, 2 * j + h0, :],
                                        start=(first and h0 == 0),
                                        stop=(last and h0 == 1),
                                        skip_group_check=True)
                                else:
                                    nc.tensor.matmul(
                                        psc[g][:, col:col + 8],
                                        et0_v[:, h0, tt * 128:(tt + 1) * 128],
                                        vts[a0 // 4][:, a0 % 4, 2 * j + h0, 0:8],
                                        start=(first and h0 == 0),
                                        stop=(last and h0 == 1),
                                        skip_group_check=True)

                for a in range(na):
                    diag = a >= 4 * b
                    d_off = 128 * (a - 4 * b) if diag else 0
                    pw = pS.tile([128, 1024], DT, tag="pss",
                                 name=f"pss{b}_{j}_{a}")
                    for h in range(2):
                        if "scores" not in ablate:
                            nc.tensor.matmul(
                                pw[:, h * 512 + d_off:(h + 1) * 512],
                                kts[a // 4][h * 64:(h + 1) * 64, j,
                                            (a % 4) * 128:(a % 4 + 1) * 128],
                                qt[h * 64:(h + 1) * 64, j, d_off:512],
                                start=True, stop=True, tile_position=(h * 64, 0))
                        else:
                            nc.tensor.matmul(
                                pw[:, h * 512:h * 512 + 8],
                                kts[a // 4][h * 64:(h + 1) * 64, j,
                                            (a % 4) * 128:(a % 4 + 1) * 128],
                                qt[h * 64:(h + 1) * 64, j, 0:8],
                                start=True, stop=True, tile_position=(h * 64, 0))
                    et = ep.tile([128, 1024], BF, tag="et", name=f"et{b}_{j}_{a}")
                    et_v = et[:].rearrange("p (h f) -> p h f", h=2)
                    pw_v = pw[:].rearrange("p (h f) -> p h f", h=2)
                    if "expdve" in ablate:
                        with nc.allow_low_precision(reason="ablation"):
                            nc.vector.tensor_copy(et[:], pw[:])
                    else:
                        nc.scalar.activation(
                            et_v[:, :, d_off:512], pw_v[:, :, d_off:512],
                            EXP, scale=0.125)
                    if diag and "mask" not in ablate:
                        with nc.allow_low_precision(reason="bf16 mask mul"):
                            nc.vector.tensor_mul(
                                et_v[:, :, d_off:d_off + 128],
                                et_v[:, :, d_off:d_off + 128],
                                tri[:])
                    pend.append((a, d_off, et))
                    flush(flush_depth)
                flush(0)
                # normalize: 1/Z from psum col 64/129, fused mul -> c-hat bf16
                for tt in range(4):
                    pg = psc[tt // 2]
                    zr = zp.tile([128, 2], FR, tag="zr", name=f"zr{b}_{j}_{tt}")
                    z_ap = bass.AP(
                        tensor=pg.tensor, offset=pg.offset + (tt % 2) * 130 + 64,
                        ap=[list(pg.ap[0]), [65, 2]])
                    with nc.allow_low_precision(reason="f32r recip"):
                        nc.vector.reciprocal(zr[:], z_ap)
                    c_ap = bass.AP(
                        tensor=pg.tensor, offset=pg.offset + (tt % 2) * 130,
                        ap=[list(pg.ap[0]), [65, 2], [1, 64]])
                    zb_ap = bass.AP(
                        tensor=zr.tensor, offset=zr.offset,
                        ap=[list(zr.ap[0]), [1, 2], [0, 64]])
                    ch = chs[tt]
                    ch_ap = bass.AP(
                        tensor=ch.tensor, offset=ch.offset + j * 128,
                        ap=[list(ch.ap[0]), [64, 2], [1, 64]])
                    with nc.allow_low_precision(reason="bf16 store"):
                        nc.vector.tensor_mul(ch_ap, c_ap, zb_ap)
            # transpose c-hat [t, c'] -> cx [c', t] via XBAR DMA
            for tt in range(4):
                cx_slice = cx[:, :, tt * 128:(tt + 1) * 128]
                nc.scalar.dma_start_transpose(cx_slice, chs[tt][:])

        def outproj(r, ts_list=range(4)):
            cx = cxs[r]
            for ts in ts_list:
                ot = op.tile([128, D], BF, tag="ot", name=f"ot{r}_{ts}")
                ps = pS.tile([128, D], DT, tag="pss", name=f"pso{r}_{ts}")
                for oh in range(2):
                    if "outproj" not in ablate:
                        for j in range(NJ):
                            nc.tensor.matmul(
                                ps[:, oh * 512:(oh + 1) * 512],
                                cx[:, j, ts * 128:(ts + 1) * 128],
                                wo_s[:, j, oh * 512:(oh + 1) * 512],
                                start=(j == 0), stop=(j == NJ - 1))
                    else:
                        nc.tensor.matmul(
                            ps[:, oh * 512:oh * 512 + 8],
                            cx[:, 0, ts * 128:(ts + 1) * 128],
                            wo_s[:, 0, oh * 512:oh * 512 + 8],
                            start=True, stop=True)
                with nc.allow_low_precision(reason="bf16 partial output"):
                    nc.vector.tensor_copy(ot[:], ps[:])
                if "outdma" not in ablate:
                    nc.sync.dma_start(
                        out_d[(r * 4 + ts) * 128:(r * 4 + ts + 1) * 128, :], ot[:])

        proj(0)
        for r in range(NR):
            attn(r)
            if r + 1 < NR:
                proj(r + 1)
            if r == NR - 1:
                if NR >= 2:
                    outproj(NR - 2)
                outproj(r)
            elif r != NR - 2:
                outproj(r)

    nc.compile()
    return nc


def make_in_maps(x, W_q, W_k, W_v, W_o):
    T = x.shape[1]
    NR = T // 512
    in_maps = []
    for core in range(8):
        b, g = core // 2, core % 2
        sl = slice(g * C, (g + 1) * C)
        xT = np.ascontiguousarray(x[b].T)
        xr = np.ascontiguousarray(
            xT.reshape(NKC, 128, NR, 512).transpose(1, 2, 0, 3))
        in_maps.append({
            "xr": xr.astype(ml_dtypes.bfloat16),
            "wq": np.ascontiguousarray(
                W_q[:, sl].reshape(NKC, 128, C).transpose(1, 0, 2)
            ).astype(ml_dtypes.bfloat16),
            "wk": np.ascontiguousarray(
                W_k[:, sl].reshape(NKC, 128, C).transpose(1, 0, 2)
            ).astype(ml_dtypes.bfloat16),
            "wv": np.ascontiguousarray(
                W_v[:, sl].reshape(NKC, 128, C).transpose(1, 0, 2)
            ).astype(ml_dtypes.bfloat16),
            "wo": np.ascontiguousarray(
                W_o[sl, :].reshape(NJ, 128, D).transpose(1, 0, 2)
            ).astype(ml_dtypes.bfloat16),
        })
    return in_maps


_NC_CACHE = {}


def kernel(x, W_q, W_k, W_v, W_o):
    x = np.asarray(x, dtype=np.float32)
    W_q = np.asarray(W_q, dtype=np.float32)
    W_k = np.asarray(W_k, dtype=np.float32)
    W_v = np.asarray(W_v, dtype=np.float32)
    W_o = np.asarray(W_o, dtype=np.float32)
    T = x.shape[1]
    if T not in _NC_CACHE:
        _NC_CACHE[T] = build_nc(T)
    nc = _NC_CACHE[T]
    res = run_bass_kernel_spmd(nc, make_in_maps(x, W_q, W_k, W_v, W_o),
                               list(range(8))).results
    out = np.stack([res[2 * b]["out"].astype(np.float32)
                    + res[2 * b + 1]["out"].astype(np.float32)
                    for b in range(4)])
    return out.astype(np.float32)
